# revision 2
# baseline (speedup 1.0000x reference)
"""Quantized int8 matmul on 8 TRN2 NeuronCores.

Math: out = ((x - ZP_X) * SCALE_X) @ ((y - ZP_Y) * SCALE_Y)
Implemented as: out = [(x - ZP_X) @ (y - ZP_Y)] * (SCALE_X * SCALE_Y)
The zero-point-shifted int8 values (range ~[-150, 155]) are exactly
representable in bf16, so a bf16 matmul with fp32 PSUM accumulation is
numerically ~identical to the fp32 reference.

Sharding: x row-sharded (M) across 8 cores, y replicated, no collectives.
Each core's x shard is laid out [K, m_loc] in DRAM (layout chosen at
shard time on host) so the TensorE stationary operand [k-part, m-free]
loads directly -- no on-device transpose.

v2 structure per core (m_loc=512):
  - startup: single-ktile x/y loads + DVE converts at priority 0 so the
    first real matmul issues ~2.5us into the graded window; a short run
    of N=128 dummy matmuls fills the DMA-wait gap and starts the HAM
    warmup clock early.
  - main: 7 full n-blocks (w=512), kt-major MM order; y staged in
    [128,4,512] int8 batches (sync HWDGE) converted to bf16 on
    DVE/ACT alternately; per-block evictions merge into one
    [128,4,512] fp32 tile stored with a single 1MB DMA (sync/ACT
    HWDGE alternating).
  - tail: final n-block as two w=256 halves computed mt-major with
    per-mt immediate evict+store (128KB DMAs striped across queues)
    so the post-stream drain is one small transfer.
  - x converts run on DVE early then gpsimd/DVE, keeping ACT/DVE free
    for y; x batches 2+ load via gpsimd SWDGE to spare the sync queue.
"""

import numpy as np

SCALE_X, ZP_X = 0.0215, -25
SCALE_Y, ZP_Y = 0.0176, 18
M, K, N = 4096, 4096, 4096
N_CORES = 8
P = 128
NBLK = 512  # full n-block width = one PSUM bank of fp32
KB = 4  # k-tiles per y DMA/convert batch (full blocks)
KBH = 8  # k-tiles per y batch in the half-width tail blocks
XB = 4  # k-tiles per x DMA/convert batch
N_WARM = 12  # PE warm-up dummy matmuls (N=128 each, ~107ns cold)


def build_nc(m_loc, k, n):
    from contextlib import ExitStack

    import concourse.mybir as mybir
    import concourse.tile as tile
    from concourse import bacc
    from concourse.bass import ds, ts

    fp32 = mybir.dt.float32
    bf16 = mybir.dt.bfloat16
    int8 = mybir.dt.int8
    Copy = mybir.ActivationFunctionType.Copy
    SCALE = float(SCALE_X * SCALE_Y)

    MT = m_loc // P  # m tiles (4)
    KT = k // P  # contraction tiles (32)
    NB = n // NBLK  # n blocks (8)
    NKB = KT // KB  # y batches per full block (8)
    NXB = KT // XB  # x batches (8)

    nc = bacc.Bacc(None, debug=False)
    xt = nc.declare_dram_parameter("xt", [k, m_loc], int8, isOutput=False)
    y = nc.declare_dram_parameter("y", [k, n], int8, isOutput=False)
    out = nc.declare_dram_parameter("out", [m_loc, n], fp32, isOutput=True)

    # DRAM views
    xt_r1 = xt.rearrange("(t p) m -> t p m", p=P)  # single k-tiles
    xt_r4 = xt.rearrange("(g b p) m -> g p b m", b=XB, p=P)
    y_r1 = y.rearrange("(t p) n -> t p n", p=P)
    y_r4 = y.rearrange("(q b p) n -> q p b n", b=KB, p=P)
    y_r8 = y.rearrange("(q b p) n -> q p b n", b=KBH, p=P)
    out_pm = out.rearrange("(t p) n -> p t n", p=P)  # partition-major store

    with ExitStack() as ctx:
        tc = ctx.enter_context(tile.TileContext(nc))
        wm_pool = ctx.enter_context(tc.tile_pool(name="wm", bufs=2))
        xi_pool = ctx.enter_context(tc.tile_pool(name="xi", bufs=4))
        xt_pool = ctx.enter_context(tc.tile_pool(name="xtb", bufs=1, side="right"))
        yi_pool = ctx.enter_context(tc.tile_pool(name="yi", bufs=6))
        yb_pool = ctx.enter_context(tc.tile_pool(name="yb", bufs=12, side="right"))
        ob_pool = ctx.enter_context(tc.tile_pool(name="ob", bufs=3))
        ps_pool = ctx.enter_context(tc.tile_pool(name="ps", bufs=8, space="PSUM"))

        # Persistent bf16 x^T: partition = k within tile, free = (kt, m)
        xT = xt_pool.tile([P, KT, m_loc], bf16)

        # ---- startup: priority-0 block ----
        with tc.high_priority():
            wm = wm_pool.tile([P, NBLK], bf16)
            nc.vector.memset(wm[:], 0.0)
            # preload the ACT Copy table off the critical path
            dummy_o = wm_pool.tile([P, 1], fp32, name="dummy_o")
            nc.scalar.activation(dummy_o[:], wm[:, 0:1], Copy, scale=1.0)

            # first x k-tiles (gate LDWEIGHTS of the first matmuls)
            xs = []
            for t in range(2):
                xi = xi_pool.tile([P, m_loc], int8, name=f"xs{t}", tag="xi")
                nc.sync.dma_start(xi[:], xt_r1[t])
                xs.append(xi)
            # first y batch as single k-tiles for minimum latency
            yb0 = yb_pool.tile([P, KB, NBLK], bf16, name="yb0", tag="yb")
            ys = []
            for t in range(KB):
                yi = yi_pool.tile([P, NBLK], int8, name=f"ys{t}", tag="yi")
                nc.sync.dma_start(yi[:], y_r1[t, :, ds(0, NBLK)])
                ys.append(yi)
            for t in range(2):
                nc.vector.tensor_scalar_add(xT[:, t, :], xs[t][:], float(-ZP_X))
            for t in range(KB):
                nc.vector.tensor_scalar_add(yb0[:, t, :], ys[t][:], float(-ZP_Y))

            # PE warm-up dummies: fill the startup DMA window, start the
            # HAM activity clock.  N=128 keeps them cheap (~107ns cold).
            ps_warm = ps_pool.tile([P, P], fp32, tag="ps", name="warm")
            for _ in range(N_WARM):
                nc.tensor.matmul(ps_warm[:], wm[:, :P], wm[:, P : 2 * P],
                                 start=True, stop=True)

        def emit_x(g):
            # batch g covers k-tiles 4g..4g+3; g==0 handled at startup (k-
            # tiles 0,1) plus here (2,3)
            if g >= NXB:
                return
            if g == 0:
                for t in range(2, XB):
                    xi = xi_pool.tile([P, m_loc], int8, name=f"xs{t}", tag="xi")
                    nc.sync.dma_start(xi[:], xt_r1[t])
                    nc.vector.tensor_scalar_add(xT[:, t, :], xi[:], float(-ZP_X))
                return
            xi = xi_pool.tile([P, XB, m_loc], int8, name=f"xi_{g}", tag="xi4")
            deng = nc.sync if g == 1 else nc.gpsimd
            deng.dma_start(xi[:], xt_r4[g])
            ceng = nc.gpsimd if g % 2 == 0 else nc.vector
            ceng.tensor_scalar_add(xT[:, ts(g, XB), :], xi[:], float(-ZP_X))

        # ---- main loop: full-width blocks 0..NB-2 ----
        for bi in range(NB - 1):
            col = bi * NBLK
            psums = [
                ps_pool.tile([P, NBLK], fp32, tag="ps", name=f"acc_{bi}_{i}")
                for i in range(MT)
            ]
            if bi == 0:
                emit_x(0)
                emit_x(1)
            for q in range(NKB):
                if bi == 0 and q > 0:
                    emit_x(q + 1)
                if bi == 0 and q == 0:
                    yb = yb0
                else:
                    yi = yi_pool.tile([P, KB, NBLK], int8, name=f"yi_{bi}_{q}",
                                      tag="yi")
                    nc.sync.dma_start(yi[:], y_r4[q, :, :, ds(col, NBLK)])
                    yb = yb_pool.tile([P, KB, NBLK], bf16, name=f"yb_{bi}_{q}",
                                      tag="yb")
                    if q % 2 == 0:
                        nc.vector.tensor_scalar_add(yb[:], yi[:], float(-ZP_Y))
                    else:
                        nc.scalar.activation(yb[:], yi[:], Copy, bias=float(-ZP_Y))
                for kti in range(KB):
                    kt = q * KB + kti
                    for mt in range(MT):
                        nc.tensor.matmul(
                            psums[mt][:],
                            xT[:, kt, ts(mt, P)],
                            yb[:, kti, :],
                            start=(kt == 0),
                            stop=(kt == KT - 1),
                        )
            # merged eviction: 4 psum tiles -> one [P, MT, NBLK] tile,
            # one 1MB store
            ob = ob_pool.tile([P, MT, NBLK], fp32, name=f"ob_{bi}", tag="ob")
            for mt in range(MT):
                if mt % 2 == 0:
                    nc.scalar.activation(ob[:, mt, :], psums[mt][:], Copy,
                                         scale=SCALE)
                else:
                    nc.vector.tensor_scalar_mul(ob[:, mt, :], psums[mt][:], SCALE)
            oeng = nc.sync if bi % 2 == 0 else nc.scalar
            oeng.dma_start(out_pm[:, :, ds(col, NBLK)], ob[:])

        # ---- tail: last block as two w=256 halves, mt-major ----
        half = NBLK // 2
        for h in range(2):
            col = (NB - 1) * NBLK + h * half
            ybs = []
            for q in range(KT // KBH):
                yi = yi_pool.tile([P, KBH, half], int8, name=f"yih_{h}_{q}",
                                  tag="yi")
                nc.sync.dma_start(yi[:], y_r8[q, :, :, ds(col, half)])
                yb = yb_pool.tile([P, KBH, half], bf16, name=f"ybh_{h}_{q}",
                                  tag="yb")
                if q % 2 == 0:
                    nc.vector.tensor_scalar_add(yb[:], yi[:], float(-ZP_Y))
                else:
                    nc.scalar.activation(yb[:], yi[:], Copy, bias=float(-ZP_Y))
                ybs.append(yb)
            psums = [
                ps_pool.tile([P, half], fp32, tag="ps", name=f"acch_{h}_{i}")
                for i in range(MT)
            ]
            for mt in range(MT):
                for q in range(KT // KBH):
                    for kti in range(KBH):
                        kt = q * KBH + kti
                        nc.tensor.matmul(
                            psums[mt][:],
                            xT[:, kt, ts(mt, P)],
                            ybs[q][:, kti, :],
                            start=(kt == 0),
                            stop=(kt == KT - 1),
                        )
                obh = ob_pool.tile([P, half], fp32, name=f"obh_{h}_{mt}",
                                   tag="ob")
                if mt % 2 == 0:
                    nc.scalar.activation(obh[:], psums[mt][:], Copy, scale=SCALE)
                else:
                    nc.vector.tensor_scalar_mul(obh[:], psums[mt][:], SCALE)
                oeng = nc.scalar if mt % 2 == 0 else nc.sync
                oeng.dma_start(out[ts(mt, P), ds(col, half)], obh[:])

    nc.compile()
    return nc


_NC_CACHE = None
LAST_RESULT = None  # BassKernelResults of the most recent run (for profiling)


def _ensure_ntff_hook():
    """concourse's trace path imports antenv.axon_hooks, which is absent
    from this container's antenv stub. Provide it (with the real libaxon
    ctypes hook when available) so tracing works -- or degrades cleanly."""
    import sys
    import types

    try:
        import antenv.axon_hooks  # noqa: F401

        return
    except ImportError:
        pass
    mod = types.ModuleType("antenv.axon_hooks")
    holder = [None]
    mod.set_axon_ntff_profile_hook = lambda h: holder.__setitem__(0, h)
    mod.get_axon_ntff_profile_hook = lambda: holder[0]
    sys.modules["antenv.axon_hooks"] = mod
    try:
        import antenv

        antenv.axon_hooks = mod
    except ImportError:
        pass
    try:
        from trn_agent_boot.trn_boot import _ntff_profile_via_ctypes

        mod.set_axon_ntff_profile_hook(
            _ntff_profile_via_ctypes("/opt/axon/libaxon_pjrt.so")
        )
    except Exception:
        pass  # no hook -> concourse logs a warning and skips tracing


def kernel(x, y):
    global _NC_CACHE, LAST_RESULT
    _ensure_ntff_hook()
    from concourse.bass_utils import run_bass_kernel_spmd

    x = np.asarray(x)
    y = np.asarray(y)
    assert x.shape == (M, K) and y.shape == (K, N), (x.shape, y.shape)
    x8 = x.astype(np.int8) if x.dtype != np.int8 else x
    y8 = y.astype(np.int8) if y.dtype != np.int8 else y

    if _NC_CACHE is None:
        _NC_CACHE = build_nc(M // N_CORES, K, N)
    nc = _NC_CACHE

    m_loc = M // N_CORES
    in_maps = [
        {
            "xt": np.ascontiguousarray(x8[i * m_loc : (i + 1) * m_loc].T),
            "y": y8,
        }
        for i in range(N_CORES)
    ]
    res = run_bass_kernel_spmd(nc, in_maps, core_ids=list(range(N_CORES)))
    LAST_RESULT = res
    return np.concatenate(
        [np.asarray(res.results[i]["out"]) for i in range(N_CORES)], axis=0
    )


# revision 3
# speedup vs baseline: 1.3043x; 1.3043x over previous
"""Quantized int8 matmul on 8 TRN2 NeuronCores.

Math: out = ((x - ZP_X) * SCALE_X) @ ((y - ZP_Y) * SCALE_Y)
Implemented as: out = [(x - ZP_X) @ (y - ZP_Y)] * (SCALE_X * SCALE_Y)
The zero-point-shifted int8 values (range ~[-150, 155]) are exactly
representable in bf16, so a bf16 matmul with fp32 PSUM accumulation is
numerically ~identical to the fp32 reference.

Sharding: x row-sharded (M) across 8 cores, y replicated, no collectives.
Each core's x shard is laid out [K, m_loc] in DRAM (layout chosen at
shard time on host) so the TensorE stationary operand [k-part, m-free]
loads directly -- no on-device transpose.

v2 structure per core (m_loc=512):
  - startup: single-ktile x/y loads + DVE converts at priority 0 so the
    first real matmul issues ~2.5us into the graded window; a short run
    of N=128 dummy matmuls fills the DMA-wait gap and starts the HAM
    warmup clock early.
  - main: 7 full n-blocks (w=512), kt-major MM order; y staged in
    [128,4,512] int8 batches (sync HWDGE) converted to bf16 on
    DVE/ACT alternately; per-block evictions merge into one
    [128,4,512] fp32 tile stored with a single 1MB DMA (sync/ACT
    HWDGE alternating).
  - tail: final n-block as two w=256 halves computed mt-major with
    per-mt immediate evict+store (128KB DMAs striped across queues)
    so the post-stream drain is one small transfer.
  - x converts run on DVE early then gpsimd/DVE, keeping ACT/DVE free
    for y; x batches 2+ load via gpsimd SWDGE to spare the sync queue.
"""

import numpy as np

SCALE_X, ZP_X = 0.0215, -25
SCALE_Y, ZP_Y = 0.0176, 18
M, K, N = 4096, 4096, 4096
N_CORES = 8
P = 128
NBLK = 512  # full n-block width = one PSUM bank of fp32
KB = 4  # k-tiles per y DMA/convert batch (full blocks)
KBH = 8  # k-tiles per y batch in the half-width tail blocks
XB = 4  # k-tiles per x DMA/convert batch
N_WARM = 12  # PE warm-up dummy matmuls (N=128 each, ~107ns cold)


def build_nc(m_loc, k, n):
    from contextlib import ExitStack

    import concourse.mybir as mybir
    import concourse.tile as tile
    from concourse import bacc
    from concourse.bass import ds, ts

    fp32 = mybir.dt.float32
    bf16 = mybir.dt.bfloat16
    int8 = mybir.dt.int8
    Copy = mybir.ActivationFunctionType.Copy
    SCALE = float(SCALE_X * SCALE_Y)

    MT = m_loc // P  # m tiles (4)
    KT = k // P  # contraction tiles (32)
    NB = n // NBLK  # n blocks (8)
    NKB = KT // KB  # y batches per full block (8)
    NXB = KT // XB  # x batches (8)

    nc = bacc.Bacc(None, debug=False)
    xt = nc.declare_dram_parameter("xt", [k, m_loc], int8, isOutput=False)
    y = nc.declare_dram_parameter("y", [k, n], int8, isOutput=False)
    out = nc.declare_dram_parameter("out", [m_loc, n], fp32, isOutput=True)

    # DRAM views
    xt_r1 = xt.rearrange("(t p) m -> t p m", p=P)  # single k-tiles
    xt_r4 = xt.rearrange("(g b p) m -> g p b m", b=XB, p=P)
    y_r1 = y.rearrange("(t p) n -> t p n", p=P)
    y_r4 = y.rearrange("(q b p) n -> q p b n", b=KB, p=P)
    y_r8 = y.rearrange("(q b p) n -> q p b n", b=KBH, p=P)
    out_pm = out.rearrange("(t p) n -> p t n", p=P)  # partition-major store

    with ExitStack() as ctx:
        tc = ctx.enter_context(tile.TileContext(nc))
        wm_pool = ctx.enter_context(tc.tile_pool(name="wm", bufs=2))
        xi_pool = ctx.enter_context(tc.tile_pool(name="xi", bufs=4))
        xt_pool = ctx.enter_context(tc.tile_pool(name="xtb", bufs=1, side="right"))
        yi_pool = ctx.enter_context(tc.tile_pool(name="yi", bufs=6))
        yb_pool = ctx.enter_context(tc.tile_pool(name="yb", bufs=12, side="right"))
        ob_pool = ctx.enter_context(tc.tile_pool(name="ob", bufs=3))
        ps_pool = ctx.enter_context(tc.tile_pool(name="ps", bufs=8, space="PSUM"))

        # Persistent bf16 x^T: partition = k within tile, free = (kt, m)
        xT = xt_pool.tile([P, KT, m_loc], bf16)

        # ---- startup: priority-0 block ----
        with tc.high_priority():
            wm = wm_pool.tile([P, NBLK], bf16)
            nc.vector.memset(wm[:], 0.0)
            # preload the ACT Copy table off the critical path
            dummy_o = wm_pool.tile([P, 1], fp32, name="dummy_o")
            nc.scalar.activation(dummy_o[:], wm[:, 0:1], Copy, scale=1.0)

            # first x k-tiles (gate LDWEIGHTS of the first matmuls)
            xs = []
            for t in range(2):
                xi = xi_pool.tile([P, m_loc], int8, name=f"xs{t}", tag="xi")
                nc.sync.dma_start(xi[:], xt_r1[t])
                xs.append(xi)
            # first y batch as single k-tiles for minimum latency
            yb0 = yb_pool.tile([P, KB, NBLK], bf16, name="yb0", tag="yb")
            ys = []
            for t in range(KB):
                yi = yi_pool.tile([P, NBLK], int8, name=f"ys{t}", tag="yi")
                nc.sync.dma_start(yi[:], y_r1[t, :, ds(0, NBLK)])
                ys.append(yi)
            for t in range(2):
                nc.vector.tensor_scalar_add(xT[:, t, :], xs[t][:], float(-ZP_X))
            for t in range(KB):
                nc.vector.tensor_scalar_add(yb0[:, t, :], ys[t][:], float(-ZP_Y))

            # PE warm-up dummies: fill the startup DMA window, start the
            # HAM activity clock.  N=128 keeps them cheap (~107ns cold).
            ps_warm = ps_pool.tile([P, P], fp32, tag="ps", name="warm")
            for _ in range(N_WARM):
                nc.tensor.matmul(ps_warm[:], wm[:, :P], wm[:, P : 2 * P],
                                 start=True, stop=True)

        def emit_x(g):
            # batch g covers k-tiles 4g..4g+3; g==0 handled at startup (k-
            # tiles 0,1) plus here (2,3)
            if g >= NXB:
                return
            if g == 0:
                for t in range(2, XB):
                    xi = xi_pool.tile([P, m_loc], int8, name=f"xs{t}", tag="xi")
                    nc.sync.dma_start(xi[:], xt_r1[t])
                    nc.vector.tensor_scalar_add(xT[:, t, :], xi[:], float(-ZP_X))
                return
            xi = xi_pool.tile([P, XB, m_loc], int8, name=f"xi_{g}", tag="xi4")
            deng = nc.sync if g == 1 else nc.gpsimd
            deng.dma_start(xi[:], xt_r4[g])
            if g % 2 == 0:
                nc.scalar.activation(xT[:, ts(g, XB), :], xi[:], Copy,
                                     bias=float(-ZP_X))
            else:
                nc.vector.tensor_scalar_add(xT[:, ts(g, XB), :], xi[:],
                                            float(-ZP_X))

        # ---- main loop: full-width blocks 0..NB-2 ----
        for bi in range(NB - 1):
            col = bi * NBLK
            psums = [
                ps_pool.tile([P, NBLK], fp32, tag="ps", name=f"acc_{bi}_{i}")
                for i in range(MT)
            ]
            if bi == 0:
                emit_x(0)
                emit_x(1)
            for q in range(NKB):
                if bi == 0 and q > 0:
                    emit_x(q + 1)
                if bi == 0 and q == 0:
                    yb = yb0
                else:
                    yi = yi_pool.tile([P, KB, NBLK], int8, name=f"yi_{bi}_{q}",
                                      tag="yi")
                    nc.sync.dma_start(yi[:], y_r4[q, :, :, ds(col, NBLK)])
                    yb = yb_pool.tile([P, KB, NBLK], bf16, name=f"yb_{bi}_{q}",
                                      tag="yb")
                    if q % 2 == 0:
                        nc.vector.tensor_scalar_add(yb[:], yi[:], float(-ZP_Y))
                    else:
                        nc.scalar.activation(yb[:], yi[:], Copy, bias=float(-ZP_Y))
                for kti in range(KB):
                    kt = q * KB + kti
                    for mt in range(MT):
                        nc.tensor.matmul(
                            psums[mt][:],
                            xT[:, kt, ts(mt, P)],
                            yb[:, kti, :],
                            start=(kt == 0),
                            stop=(kt == KT - 1),
                        )
            # merged eviction: 4 psum tiles -> one [P, MT, NBLK] tile,
            # one 1MB store
            ob = ob_pool.tile([P, MT, NBLK], fp32, name=f"ob_{bi}", tag="ob")
            for mt in range(MT):
                if mt % 2 == 0:
                    nc.scalar.activation(ob[:, mt, :], psums[mt][:], Copy,
                                         scale=SCALE)
                else:
                    nc.vector.tensor_scalar_mul(ob[:, mt, :], psums[mt][:], SCALE)
            oeng = nc.sync if bi % 2 == 0 else nc.scalar
            oeng.dma_start(out_pm[:, :, ds(col, NBLK)], ob[:])

        # ---- tail: last block as two w=256 halves, mt-major ----
        half = NBLK // 2
        for h in range(2):
            col = (NB - 1) * NBLK + h * half
            ybs = []
            for q in range(KT // KBH):
                yi = yi_pool.tile([P, KBH, half], int8, name=f"yih_{h}_{q}",
                                  tag="yi")
                nc.sync.dma_start(yi[:], y_r8[q, :, :, ds(col, half)])
                yb = yb_pool.tile([P, KBH, half], bf16, name=f"ybh_{h}_{q}",
                                  tag="yb")
                if q % 2 == 0:
                    nc.vector.tensor_scalar_add(yb[:], yi[:], float(-ZP_Y))
                else:
                    nc.scalar.activation(yb[:], yi[:], Copy, bias=float(-ZP_Y))
                ybs.append(yb)
            psums = [
                ps_pool.tile([P, half], fp32, tag="ps", name=f"acch_{h}_{i}")
                for i in range(MT)
            ]
            for mt in range(MT):
                for q in range(KT // KBH):
                    for kti in range(KBH):
                        kt = q * KBH + kti
                        nc.tensor.matmul(
                            psums[mt][:],
                            xT[:, kt, ts(mt, P)],
                            ybs[q][:, kti, :],
                            start=(kt == 0),
                            stop=(kt == KT - 1),
                        )
                obh = ob_pool.tile([P, half], fp32, name=f"obh_{h}_{mt}",
                                   tag="ob")
                if mt % 2 == 0:
                    nc.scalar.activation(obh[:], psums[mt][:], Copy, scale=SCALE)
                else:
                    nc.vector.tensor_scalar_mul(obh[:], psums[mt][:], SCALE)
                oeng = nc.scalar if mt % 2 == 0 else nc.sync
                oeng.dma_start(out[ts(mt, P), ds(col, half)], obh[:])

    nc.compile()
    return nc


_NC_CACHE = None
LAST_RESULT = None  # BassKernelResults of the most recent run (for profiling)


def _ensure_ntff_hook():
    """concourse's trace path imports antenv.axon_hooks, which is absent
    from this container's antenv stub. Provide it (with the real libaxon
    ctypes hook when available) so tracing works -- or degrades cleanly."""
    import sys
    import types

    try:
        import antenv.axon_hooks  # noqa: F401

        return
    except ImportError:
        pass
    mod = types.ModuleType("antenv.axon_hooks")
    holder = [None]
    mod.set_axon_ntff_profile_hook = lambda h: holder.__setitem__(0, h)
    mod.get_axon_ntff_profile_hook = lambda: holder[0]
    sys.modules["antenv.axon_hooks"] = mod
    try:
        import antenv

        antenv.axon_hooks = mod
    except ImportError:
        pass
    try:
        from trn_agent_boot.trn_boot import _ntff_profile_via_ctypes

        mod.set_axon_ntff_profile_hook(
            _ntff_profile_via_ctypes("/opt/axon/libaxon_pjrt.so")
        )
    except Exception:
        pass  # no hook -> concourse logs a warning and skips tracing


def kernel(x, y):
    global _NC_CACHE, LAST_RESULT
    _ensure_ntff_hook()
    from concourse.bass_utils import run_bass_kernel_spmd

    x = np.asarray(x)
    y = np.asarray(y)
    assert x.shape == (M, K) and y.shape == (K, N), (x.shape, y.shape)
    x8 = x.astype(np.int8) if x.dtype != np.int8 else x
    y8 = y.astype(np.int8) if y.dtype != np.int8 else y

    if _NC_CACHE is None:
        _NC_CACHE = build_nc(M // N_CORES, K, N)
    nc = _NC_CACHE

    m_loc = M // N_CORES
    in_maps = [
        {
            "xt": np.ascontiguousarray(x8[i * m_loc : (i + 1) * m_loc].T),
            "y": y8,
        }
        for i in range(N_CORES)
    ]
    res = run_bass_kernel_spmd(nc, in_maps, core_ids=list(range(N_CORES)))
    LAST_RESULT = res
    return np.concatenate(
        [np.asarray(res.results[i]["out"]) for i in range(N_CORES)], axis=0
    )
